# revision 15
# baseline (speedup 1.0000x reference)
"""Single-head attention on 8 Trainium2 NeuronCores.

Sharding: core c handles batch b = c//2, query half h = c%2 (2048 queries,
all 4096 keys). Host passes x^T in bf16 with each core's own query tokens
reordered to columns 0..2047 so the SPMD program is identical on all cores
(attention is permutation-invariant over keys).

Device pipeline per core:
  1. proj pass1: stationary [Wv|Wk] (128 cols) over all 4096 tokens
     -> V^T on PSUM partitions 0-63, K^T on 64-127 (full PE array rate).
     pass2: stationary [Wq|Wq] over my 2048 tokens -> Q^T duplicated on
     both partition halves (needed for row-packed score matmuls).
  2. K^T duplicated to partitions 0-63 via SBUF->SBUF DMA.
     V^T PE-transposed to V natural layout [tok,64], multiplied by the key
     mask, and a mask-valued ones-column appended -> V' [tok,65]. The ones
     column makes the PV matmul also produce softmax denominators, and
     zeroing masked rows of V' is exactly equivalent to -inf score masking.
  3. Flash loop over (q-block 512) x (k-chunk pair 256): two row-packed
     score matmuls (contraction e=64 in PE rows 0-63 / 64-127 concurrently)
     -> one wide exp on ScalarE (scale=1/sqrt(64) folded in, PSUM->SBUF
     bf16) -> two PV matmuls accumulating out^T [65, 512] in PSUM.
  4. Normalize: PE-transpose out^T chunks, DVE reciprocal of the sums
     column, multiply, DMA out.
"""

import sys

if "/opt/trn_rl_repo" not in sys.path:
    sys.path.insert(0, "/opt/trn_rl_repo")

import ml_dtypes
import numpy as np

import concourse.bass as bass
import concourse.mybir as mybir
import concourse.tile as tile
from concourse.bass_utils import run_bass_kernel_spmd
from concourse.masks import make_identity

BF16 = mybir.dt.bfloat16
F32 = mybir.dt.float32
bf16 = ml_dtypes.bfloat16

B, S, D, E = 4, 4096, 1024, 64
SH = S // 2          # per-core query count
ND = D // 128        # d chunks
NK = S // 128        # key chunks
NQB = SH // 512      # query blocks
EV = E + 1           # V' columns (V | mask-ones)

LAST_EXEC_NS = None


def _split_multi_waits(nc, max_waits=1):
    """walrus in this container rejects instructions with >1 sync wait;
    hoist extra waits onto same-engine NOPs inserted just before."""
    for bb in nc.main_func.blocks:
        insts = bb.instructions
        out = []
        changed = False
        for inst in insts:
            si = inst.sync_info
            if si is not None and len(si.on_wait) > max_waits:
                waits = list(si.on_wait)
                extra, keep = waits[:-max_waits], waits[-max_waits:]
                for w in extra:
                    out.append(
                        mybir.InstNoOp(
                            name=nc.get_next_instruction_name(),
                            engine=inst.engine,
                            sync_info=mybir.SyncInfo(on_wait=[w], on_update=[]),
                        )
                    )
                inst.sync_info = mybir.SyncInfo(
                    on_wait=keep, on_update=list(si.on_update)
                )
                changed = True
            out.append(inst)
        if changed:
            bb.instructions = out


def _build():
    nc = bass.Bass("TRN2", target_bir_lowering=False, debug=False, num_devices=8)

    xt_ext = nc.declare_dram_parameter("xt", [D, S], BF16, isOutput=False)
    # host-swizzled: [128, ND*128], wvk[p, d*128+j] = Wvk[d*128+p, j]
    wvk_ext = nc.declare_dram_parameter("wvk", [128, ND * 128], BF16, isOutput=False)
    wqq_ext = nc.declare_dram_parameter("wqq", [128, ND * 128], BF16, isOutput=False)
    bvk_ext = nc.declare_dram_parameter("bvk", [128, 1], F32, isOutput=False)
    bqq_ext = nc.declare_dram_parameter("bqq", [128, 1], F32, isOutput=False)
    maskv_ext = nc.declare_dram_parameter("maskv", [128, NK], F32, isOutput=False)
    out_ext = nc.declare_dram_parameter("out", [SH, E], F32, isOutput=True)

    AT = mybir.ActivationFunctionType
    ALU = mybir.AluOpType

    with tile.TileContext(nc) as tc:
        with (
            tc.tile_pool(name="const", bufs=1) as cpool,
            tc.tile_pool(name="big", bufs=1) as bigpool,
            tc.tile_pool(name="work", bufs=3) as wpool,
            tc.tile_pool(name="nrm", bufs=2) as npool,
            tc.tile_pool(name="ps_a", bufs=2, space="PSUM") as ps_a,
            tc.tile_pool(name="ps_s", bufs=2, space="PSUM") as ps_s,
            tc.tile_pool(name="ps_o", bufs=2, space="PSUM") as ps_o,
        ):
            # ---- constants ----
            # weights arrive host-swizzled: [128, ND*128], chunk d at cols
            # d*128:(d+1)*128 — one contiguous-per-partition DMA each
            wvk_all = cpool.tile([128, ND * 128], BF16, tag="wvk")
            nc.sync.dma_start(out=wvk_all[:], in_=wvk_ext[:])
            wqq_all = cpool.tile([128, ND * 128], BF16, tag="wqq")
            nc.sync.dma_start(out=wqq_all[:], in_=wqq_ext[:])
            wvk_sb = [wvk_all[:, d * 128 : (d + 1) * 128] for d in range(ND)]
            wqq_sb = [wqq_all[:, d * 128 : (d + 1) * 128] for d in range(ND)]
            # ---- x^T slab: [128, ND*4096] bf16 ----
            # my half (tokens 0:2048) in quarter-chunks so the first proj
            # token-blocks' deps land early; other half in half-chunks
            xt_sb = bigpool.tile([128, ND * S], BF16, tag="xt")
            for q4 in range(2):
                for d in range(ND):
                    o = q4 * 1024
                    nc.sync.dma_start(
                        out=xt_sb[:, d * S + o : d * S + o + 1024],
                        in_=xt_ext[d * 128 : (d + 1) * 128, o : o + 1024],
                    )
            bvk_sb = cpool.tile([128, 1], F32, tag="bvk")
            nc.sync.dma_start(out=bvk_sb[:], in_=bvk_ext[:])
            bqq_sb = cpool.tile([128, 1], F32, tag="bqq")
            nc.sync.dma_start(out=bqq_sb[:], in_=bqq_ext[:])
            maskv_sb = cpool.tile([128, NK], F32, tag="maskv")
            nc.sync.dma_start(out=maskv_sb[:], in_=maskv_ext[:])
            for d in range(ND):
                nc.sync.dma_start(
                    out=xt_sb[:, d * S + SH : d * S + S],
                    in_=xt_ext[d * 128 : (d + 1) * 128, SH:S],
                )
            id64 = cpool.tile([64, 64], BF16, tag="id64")
            make_identity(nc, id64[:])
            id65 = cpool.tile([65, 65], F32, tag="id65")
            make_identity(nc, id65[:])

            Q2 = bigpool.tile([128, SH], BF16, tag="q2")
            K2T = bigpool.tile([128, S], BF16, tag="k2t")
            VT = bigpool.tile([64, S], BF16, tag="vt")
            V_all = bigpool.tile([128, NK * EV], BF16, tag="vall")

            ones_col = V_all[:].rearrange("p (c e) -> p c e", e=EV)[:, :, E]
            nc.vector.tensor_copy(ones_col, maskv_sb[:])

            # ---- flash stage / normalize emitters ----
            pso_tiles = {}
            stage_done = set()

            def emit_stage(pr, qb):
                if (pr, qb) in stage_done:
                    return
                stage_done.add((pr, qb))
                if qb not in pso_tiles:
                    pso_tiles[qb] = ps_o.tile(
                        [EV, 512], F32, tag="o", name=f"pso{qb}"
                    )
                pso = pso_tiles[qb]
                qsl = slice(qb * 512, (qb + 1) * 512)
                kA, kB = 2 * pr, 2 * pr + 1
                S2 = ps_s.tile([128, 1024], F32, tag="s")
                nc.tensor.matmul(
                    S2[:, 0:512],
                    K2T[0:64, kA * 128 : (kA + 1) * 128],
                    Q2[0:64, qsl],
                    start=True,
                    stop=True,
                )
                nc.tensor.matmul(
                    S2[:, 512:1024],
                    K2T[64:128, kB * 128 : (kB + 1) * 128],
                    Q2[64:128, qsl],
                    start=True,
                    stop=True,
                )
                PT = wpool.tile([128, 1024], BF16, tag="pt", bufs=4)
                nc.scalar.activation(PT[:], S2[:], AT.Exp, bias=0.0, scale=0.125)
                nc.tensor.matmul(
                    pso[:],
                    V_all[:, kA * EV : (kA + 1) * EV],
                    PT[:, 0:512],
                    start=(pr == 0),
                    stop=False,
                    skip_group_check=True,
                )
                nc.tensor.matmul(
                    pso[:],
                    V_all[:, kB * EV : (kB + 1) * EV],
                    PT[:, 512:1024],
                    start=False,
                    stop=(pr == NK // 2 - 1),
                    skip_group_check=True,
                )

            def emit_norm(qb):
                pso = pso_tiles[qb]
                t_out = npool.tile([EV, 512], F32, tag="tout")
                nc.vector.tensor_copy(t_out[:], pso[:])
                for c in range(4):
                    ptn = ps_a.tile([128, EV], F32, tag="a")
                    nc.tensor.transpose(
                        ptn[:], t_out[:, c * 128 : (c + 1) * 128], id65[:]
                    )
                    recip = npool.tile([128, 1], F32, tag="recip")
                    nc.vector.reciprocal(recip[:], ptn[:, E : E + 1])
                    osb = npool.tile([128, E], F32, tag="osb")
                    nc.vector.tensor_scalar(
                        osb[:], ptn[:, 0:E], recip[:], None, ALU.mult
                    )
                    r0 = qb * 512 + c * 128
                    nc.sync.dma_start(out=out_ext[r0 : r0 + 128, :], in_=osb[:])

            # ---- projections + V', with the first two q-blocks' flash
            # stages woven in so ScalarE starts early. A stage (pr, qb) is
            # emitted only once its K/V chunks (token block (2pr+1)//4) and
            # Q2 block qb are emitted — Tile deps follow trace order. ----
            for tb in range(S // 512):
                sl = slice(tb * 512, (tb + 1) * 512)
                # pass1: [Wv|Wk]
                ps = ps_a.tile([128, 512], F32, tag="a")
                for d in range(ND):
                    nc.tensor.matmul(
                        ps[:],
                        wvk_sb[d],
                        xt_sb[:, d * S + tb * 512 : d * S + (tb + 1) * 512],
                        start=(d == 0),
                        stop=(d == ND - 1),
                    )
                nc.vector.tensor_scalar(
                    VT[:, sl], ps[0:64, :], bvk_sb[0:64, :], None, ALU.add
                )
                nc.vector.tensor_scalar(
                    K2T[64:128, sl], ps[64:128, :], bvk_sb[64:128, :], None, ALU.add
                )
                # duplicate K^T onto partitions 0-63 (SBUF->SBUF DMA)
                nc.sync.dma_start(out=K2T[0:64, sl], in_=K2T[64:128, sl])
                # pass2: [Wq|Wq] (my tokens only = first half)
                if tb < SH // 512:
                    ps = ps_a.tile([128, 512], F32, tag="a")
                    for d in range(ND):
                        nc.tensor.matmul(
                            ps[:],
                            wqq_sb[d],
                            xt_sb[:, d * S + tb * 512 : d * S + (tb + 1) * 512],
                            start=(d == 0),
                            stop=(d == ND - 1),
                        )
                    nc.vector.tensor_scalar(
                        Q2[:, sl], ps[:], bqq_sb[:], None, ALU.add
                    )
                # V' for this token block (4 key chunks)
                for c in range(tb * 4, tb * 4 + 4):
                    psv = ps_a.tile([128, 64], BF16, tag="a")
                    nc.tensor.transpose(psv[:], VT[:, c * 128 : (c + 1) * 128], id64[:])
                    nc.vector.tensor_scalar(
                        V_all[:, c * EV : c * EV + E],
                        psv[:],
                        maskv_sb[:, c : c + 1],
                        None,
                        ALU.mult,
                    )
                # weave in flash stages for q-blocks 0/1 whose deps exist
                for qb in (0, 1):
                    if qb <= tb:
                        for pr in range(0, 2 * tb + 2):
                            emit_stage(pr, qb)

            # ---- remaining flash stages + normalization ----
            emit_norm(0)
            emit_norm(1)
            for qb in (2, 3):
                for pr in range(NK // 2):
                    emit_stage(pr, qb)
                emit_norm(qb)

    _split_multi_waits(nc)
    return nc


_NC_CACHE = [None]


def kernel(x, mask, Wq, bq, Wk, bk, Wv, bv, _trace=False, _tmpdir=None):
    global LAST_EXEC_NS
    x = np.asarray(x, dtype=np.float32)
    mask = np.asarray(mask)
    Wq, bq = np.asarray(Wq, np.float32), np.asarray(bq, np.float32)
    Wk, bk = np.asarray(Wk, np.float32), np.asarray(bk, np.float32)
    Wv, bv = np.asarray(Wv, np.float32), np.asarray(bv, np.float32)

    def swz(w):  # [D, 128] -> [128, ND*128]: out[p, d*128+j] = w[d*128+p, j]
        return np.ascontiguousarray(
            w.reshape(ND, 128, 128).transpose(1, 0, 2).reshape(128, ND * 128)
        ).astype(bf16)

    wvk = swz(np.concatenate([Wv, Wk], axis=1))
    wqq = swz(np.concatenate([Wq, Wq], axis=1))
    bvk = np.concatenate([bv, bk])[:, None].astype(np.float32)
    bqq = np.concatenate([bq, bq])[:, None].astype(np.float32)

    in_maps = []
    for c in range(8):
        b, h = c // 2, c % 2
        xb = x[b]  # [S, D]
        mb = mask[b].astype(np.float32)  # [S]
        if h == 1:  # my query tokens first
            order = np.concatenate([np.arange(SH, S), np.arange(0, SH)])
            xb = xb[order]
            mb = mb[order]
        xt = np.ascontiguousarray(xb.T).astype(bf16)  # [D, S]
        maskv = np.ascontiguousarray(mb.reshape(NK, 128).T).astype(np.float32)
        in_maps.append(
            {
                "xt": xt,
                "wvk": wvk,
                "wqq": wqq,
                "bvk": bvk,
                "bqq": bqq,
                "maskv": maskv,
            }
        )

    if _NC_CACHE[0] is None:
        _NC_CACHE[0] = _build()
    nc = _NC_CACHE[0]

    kwargs = {}
    if _trace:
        kwargs = dict(trace=True, tmpdir=_tmpdir)
    res = run_bass_kernel_spmd(nc, in_maps, list(range(8)), **kwargs)
    LAST_EXEC_NS = res.exec_time_ns

    out = np.empty((B, S, E), dtype=np.float32)
    for c in range(8):
        b, h = c // 2, c % 2
        out[b, h * SH : (h + 1) * SH, :] = res.results[c]["out"]
    return out


# revision 16
# speedup vs baseline: 1.0495x; 1.0495x over previous
"""Single-head attention on 8 Trainium2 NeuronCores.

Sharding: core c handles batch b = c//2, query half h = c%2 (2048 queries,
all 4096 keys). Host passes x^T in bf16 with each core's own query tokens
reordered to columns 0..2047 so the SPMD program is identical on all cores
(attention is permutation-invariant over keys).

Device pipeline per core:
  1. proj pass1: stationary [Wv|Wk] (128 cols) over all 4096 tokens
     -> V^T on PSUM partitions 0-63, K^T on 64-127 (full PE array rate).
     pass2: stationary [Wq|Wq] over my 2048 tokens -> Q^T duplicated on
     both partition halves (needed for row-packed score matmuls).
  2. K^T duplicated to partitions 0-63 via SBUF->SBUF DMA.
     V^T PE-transposed to V natural layout [tok,64], multiplied by the key
     mask, and a mask-valued ones-column appended -> V' [tok,65]. The ones
     column makes the PV matmul also produce softmax denominators, and
     zeroing masked rows of V' is exactly equivalent to -inf score masking.
  3. Flash loop over (q-block 512) x (k-chunk pair 256): two row-packed
     score matmuls (contraction e=64 in PE rows 0-63 / 64-127 concurrently)
     -> one wide exp on ScalarE (scale=1/sqrt(64) folded in, PSUM->SBUF
     bf16) -> two PV matmuls accumulating out^T [65, 512] in PSUM.
  4. Normalize: PE-transpose out^T chunks, DVE reciprocal of the sums
     column, multiply, DMA out.
"""

import sys

if "/opt/trn_rl_repo" not in sys.path:
    sys.path.insert(0, "/opt/trn_rl_repo")

import ml_dtypes
import numpy as np

import concourse.bass as bass
import concourse.mybir as mybir
import concourse.tile as tile
from concourse.bass_utils import run_bass_kernel_spmd
from concourse.masks import make_identity

BF16 = mybir.dt.bfloat16
F32 = mybir.dt.float32
bf16 = ml_dtypes.bfloat16

B, S, D, E = 4, 4096, 1024, 64
SH = S // 2          # per-core query count
ND = D // 128        # d chunks
NK = S // 128        # key chunks
NQB = SH // 512      # query blocks
EV = E + 1           # V' columns (V | mask-ones)

LAST_EXEC_NS = None


def _split_multi_waits(nc, max_waits=1):
    """walrus in this container rejects instructions with >1 sync wait;
    hoist extra waits onto same-engine NOPs inserted just before."""
    for bb in nc.main_func.blocks:
        insts = bb.instructions
        out = []
        changed = False
        for inst in insts:
            si = inst.sync_info
            if si is not None and len(si.on_wait) > max_waits:
                waits = list(si.on_wait)
                extra, keep = waits[:-max_waits], waits[-max_waits:]
                for w in extra:
                    out.append(
                        mybir.InstNoOp(
                            name=nc.get_next_instruction_name(),
                            engine=inst.engine,
                            sync_info=mybir.SyncInfo(on_wait=[w], on_update=[]),
                        )
                    )
                inst.sync_info = mybir.SyncInfo(
                    on_wait=keep, on_update=list(si.on_update)
                )
                changed = True
            out.append(inst)
        if changed:
            bb.instructions = out


def _build():
    nc = bass.Bass("TRN2", target_bir_lowering=False, debug=False, num_devices=8)

    xt_ext = nc.declare_dram_parameter("xt", [D, S], BF16, isOutput=False)
    # host-swizzled: [128, ND*128], wvk[p, d*128+j] = Wvk[d*128+p, j]
    wvk_ext = nc.declare_dram_parameter("wvk", [128, ND * 128], BF16, isOutput=False)
    wqq_ext = nc.declare_dram_parameter("wqq", [128, ND * 128], BF16, isOutput=False)
    bvk_ext = nc.declare_dram_parameter("bvk", [128, 1], F32, isOutput=False)
    bqq_ext = nc.declare_dram_parameter("bqq", [128, 1], F32, isOutput=False)
    maskv_ext = nc.declare_dram_parameter("maskv", [128, NK], F32, isOutput=False)
    out_ext = nc.declare_dram_parameter("out", [SH, E], F32, isOutput=True)

    AT = mybir.ActivationFunctionType
    ALU = mybir.AluOpType

    with tile.TileContext(nc) as tc:
        with (
            tc.tile_pool(name="const", bufs=1) as cpool,
            tc.tile_pool(name="big", bufs=1) as bigpool,
            tc.tile_pool(name="work", bufs=3) as wpool,
            tc.tile_pool(name="nrm", bufs=2) as npool,
            tc.tile_pool(name="ps_a", bufs=2, space="PSUM") as ps_a,
            tc.tile_pool(name="ps_s", bufs=2, space="PSUM") as ps_s,
            tc.tile_pool(name="ps_o", bufs=2, space="PSUM") as ps_o,
        ):
            # ---- constants ----
            # weights arrive host-swizzled: [128, ND*128], chunk d at cols
            # d*128:(d+1)*128 — one contiguous-per-partition DMA each
            wvk_all = cpool.tile([128, ND * 128], BF16, tag="wvk")
            nc.sync.dma_start(out=wvk_all[:], in_=wvk_ext[:])
            wqq_all = cpool.tile([128, ND * 128], BF16, tag="wqq")
            nc.sync.dma_start(out=wqq_all[:], in_=wqq_ext[:])
            wvk_sb = [wvk_all[:, d * 128 : (d + 1) * 128] for d in range(ND)]
            wqq_sb = [wqq_all[:, d * 128 : (d + 1) * 128] for d in range(ND)]
            bvk_sb = cpool.tile([128, 1], F32, tag="bvk")
            nc.sync.dma_start(out=bvk_sb[:], in_=bvk_ext[:])
            bqq_sb = cpool.tile([128, 1], F32, tag="bqq")
            nc.sync.dma_start(out=bqq_sb[:], in_=bqq_ext[:])
            maskv_sb = cpool.tile([128, NK], F32, tag="maskv")
            nc.sync.dma_start(out=maskv_sb[:], in_=maskv_ext[:])
            id64 = cpool.tile([64, 64], BF16, tag="id64")
            make_identity(nc, id64[:])
            id65 = cpool.tile([65, 65], F32, tag="id65")
            make_identity(nc, id65[:])

            # ---- x^T slab: [128, ND*4096] bf16, chunked DMA (d x half) ----
            xt_sb = bigpool.tile([128, ND * S], BF16, tag="xt")
            for hh in range(2):
                for d in range(ND):
                    nc.sync.dma_start(
                        out=xt_sb[:, d * S + hh * SH : d * S + (hh + 1) * SH],
                        in_=xt_ext[d * 128 : (d + 1) * 128, hh * SH : (hh + 1) * SH],
                    )

            Q2 = bigpool.tile([128, SH], BF16, tag="q2")
            K2T = bigpool.tile([128, S], BF16, tag="k2t")
            VT = bigpool.tile([64, S], BF16, tag="vt")
            V_all = bigpool.tile([128, NK * EV], BF16, tag="vall")

            ones_col = V_all[:].rearrange("p (c e) -> p c e", e=EV)[:, :, E]
            nc.vector.tensor_copy(ones_col, maskv_sb[:])

            # ---- flash stage / normalize emitters ----
            pso_tiles = {}
            stage_done = set()

            def emit_stage(pr, qb):
                if (pr, qb) in stage_done:
                    return
                stage_done.add((pr, qb))
                if qb not in pso_tiles:
                    pso_tiles[qb] = ps_o.tile(
                        [EV, 512], F32, tag="o", name=f"pso{qb}"
                    )
                pso = pso_tiles[qb]
                qsl = slice(qb * 512, (qb + 1) * 512)
                kA, kB = 2 * pr, 2 * pr + 1
                S2 = ps_s.tile([128, 1024], F32, tag="s")
                nc.tensor.matmul(
                    S2[:, 0:512],
                    K2T[0:64, kA * 128 : (kA + 1) * 128],
                    Q2[0:64, qsl],
                    start=True,
                    stop=True,
                )
                nc.tensor.matmul(
                    S2[:, 512:1024],
                    K2T[64:128, kB * 128 : (kB + 1) * 128],
                    Q2[64:128, qsl],
                    start=True,
                    stop=True,
                )
                PT = wpool.tile([128, 1024], BF16, tag="pt", bufs=4)
                nc.scalar.activation(PT[:], S2[:], AT.Exp, bias=0.0, scale=0.125)
                nc.tensor.matmul(
                    pso[:],
                    V_all[:, kA * EV : (kA + 1) * EV],
                    PT[:, 0:512],
                    start=(pr == 0),
                    stop=False,
                    skip_group_check=True,
                )
                nc.tensor.matmul(
                    pso[:],
                    V_all[:, kB * EV : (kB + 1) * EV],
                    PT[:, 512:1024],
                    start=False,
                    stop=(pr == NK // 2 - 1),
                    skip_group_check=True,
                )

            def emit_norm(qb):
                pso = pso_tiles[qb]
                t_out = npool.tile([EV, 512], F32, tag="tout")
                nc.vector.tensor_copy(t_out[:], pso[:])
                for c in range(4):
                    ptn = ps_a.tile([128, EV], F32, tag="a")
                    nc.tensor.transpose(
                        ptn[:], t_out[:, c * 128 : (c + 1) * 128], id65[:]
                    )
                    recip = npool.tile([128, 1], F32, tag="recip")
                    nc.vector.reciprocal(recip[:], ptn[:, E : E + 1])
                    osb = npool.tile([128, E], F32, tag="osb")
                    nc.vector.tensor_scalar(
                        osb[:], ptn[:, 0:E], recip[:], None, ALU.mult
                    )
                    r0 = qb * 512 + c * 128
                    nc.sync.dma_start(out=out_ext[r0 : r0 + 128, :], in_=osb[:])

            # ---- projections + V', with the first two q-blocks' flash
            # stages woven in so ScalarE starts early. A stage (pr, qb) is
            # emitted only once its K/V chunks (token block (2pr+1)//4) and
            # Q2 block qb are emitted — Tile deps follow trace order. ----
            for tb in range(S // 512):
                sl = slice(tb * 512, (tb + 1) * 512)
                # pass1: [Wv|Wk]
                ps = ps_a.tile([128, 512], F32, tag="a")
                for d in range(ND):
                    nc.tensor.matmul(
                        ps[:],
                        wvk_sb[d],
                        xt_sb[:, d * S + tb * 512 : d * S + (tb + 1) * 512],
                        start=(d == 0),
                        stop=(d == ND - 1),
                    )
                nc.vector.tensor_scalar(
                    VT[:, sl], ps[0:64, :], bvk_sb[0:64, :], None, ALU.add
                )
                nc.vector.tensor_scalar(
                    K2T[64:128, sl], ps[64:128, :], bvk_sb[64:128, :], None, ALU.add
                )
                # duplicate K^T onto partitions 0-63 (SBUF->SBUF DMA)
                nc.sync.dma_start(out=K2T[0:64, sl], in_=K2T[64:128, sl])
                # pass2: [Wq|Wq] (my tokens only = first half)
                if tb < SH // 512:
                    ps = ps_a.tile([128, 512], F32, tag="a")
                    for d in range(ND):
                        nc.tensor.matmul(
                            ps[:],
                            wqq_sb[d],
                            xt_sb[:, d * S + tb * 512 : d * S + (tb + 1) * 512],
                            start=(d == 0),
                            stop=(d == ND - 1),
                        )
                    nc.vector.tensor_scalar(
                        Q2[:, sl], ps[:], bqq_sb[:], None, ALU.add
                    )
                # V' for this token block (4 key chunks)
                for c in range(tb * 4, tb * 4 + 4):
                    psv = ps_a.tile([128, 64], BF16, tag="a")
                    nc.tensor.transpose(psv[:], VT[:, c * 128 : (c + 1) * 128], id64[:])
                    nc.vector.tensor_scalar(
                        V_all[:, c * EV : c * EV + E],
                        psv[:],
                        maskv_sb[:, c : c + 1],
                        None,
                        ALU.mult,
                    )
                # weave in flash stages for q-blocks 0/1 whose deps exist
                for qb in (0, 1):
                    if qb <= tb:
                        for pr in range(0, 2 * tb + 2):
                            emit_stage(pr, qb)

            # ---- remaining flash stages + normalization ----
            emit_norm(0)
            emit_norm(1)
            for qb in (2, 3):
                for pr in range(NK // 2):
                    emit_stage(pr, qb)
                emit_norm(qb)

    _split_multi_waits(nc)
    return nc


_NC_CACHE = [None]


def kernel(x, mask, Wq, bq, Wk, bk, Wv, bv, _trace=False, _tmpdir=None):
    global LAST_EXEC_NS
    x = np.asarray(x, dtype=np.float32)
    mask = np.asarray(mask)
    Wq, bq = np.asarray(Wq, np.float32), np.asarray(bq, np.float32)
    Wk, bk = np.asarray(Wk, np.float32), np.asarray(bk, np.float32)
    Wv, bv = np.asarray(Wv, np.float32), np.asarray(bv, np.float32)

    def swz(w):  # [D, 128] -> [128, ND*128]: out[p, d*128+j] = w[d*128+p, j]
        return np.ascontiguousarray(
            w.reshape(ND, 128, 128).transpose(1, 0, 2).reshape(128, ND * 128)
        ).astype(bf16)

    wvk = swz(np.concatenate([Wv, Wk], axis=1))
    wqq = swz(np.concatenate([Wq, Wq], axis=1))
    bvk = np.concatenate([bv, bk])[:, None].astype(np.float32)
    bqq = np.concatenate([bq, bq])[:, None].astype(np.float32)

    in_maps = []
    for c in range(8):
        b, h = c // 2, c % 2
        xb = x[b]  # [S, D]
        mb = mask[b].astype(np.float32)  # [S]
        if h == 1:  # my query tokens first
            order = np.concatenate([np.arange(SH, S), np.arange(0, SH)])
            xb = xb[order]
            mb = mb[order]
        xt = np.ascontiguousarray(xb.T).astype(bf16)  # [D, S]
        maskv = np.ascontiguousarray(mb.reshape(NK, 128).T).astype(np.float32)
        in_maps.append(
            {
                "xt": xt,
                "wvk": wvk,
                "wqq": wqq,
                "bvk": bvk,
                "bqq": bqq,
                "maskv": maskv,
            }
        )

    if _NC_CACHE[0] is None:
        _NC_CACHE[0] = _build()
    nc = _NC_CACHE[0]

    kwargs = {}
    if _trace:
        kwargs = dict(trace=True, tmpdir=_tmpdir)
    res = run_bass_kernel_spmd(nc, in_maps, list(range(8)), **kwargs)
    LAST_EXEC_NS = res.exec_time_ns

    out = np.empty((B, S, E), dtype=np.float32)
    for c in range(8):
        b, h = c // 2, c % 2
        out[b, h * SH : (h + 1) * SH, :] = res.results[c]["out"]
    return out
